# revision 8
# baseline (speedup 1.0000x reference)
"""Trainium2 Bass kernel for nn_CustomPatchEmbedding.

Math: per row, the int id map segments the 1376 columns into 96 segments.
Each segment becomes one patch: gather min(len, P) values (P = closest of
(5,10,17,24)), multiply by W_P.T -> [512], scatter to out[row, slot], add a
sin/cos positional embedding.  The id map produced by ``setup_inputs`` is
identical across rows and periodic: 6 segments spanning 86 columns, tiled 16
times.  That lets the whole gather + 4 bucketed GEMMs + scatter collapse into
ONE dense GEMM,

    x.reshape(B*16, 86) @ Wbig[86, 6*512]  ->  out.reshape(B, 96, 512)

where Wbig places each W_k.T block at its segment's column offset and encodes
padding/truncation as zero rows.  The structure (period, offsets, buckets) is
re-derived at runtime from the actual ``x_opath_batch`` input; if the input
turns out not to have the expected structure we fall back to a pure-numpy
computation (never triggers for the real harness inputs).

Device work per core (data-parallel over batch, 32 rows/core):
  XT = transpose(x_shard [512, 86]) via PE;  out = XT.T @ Wbig + posemb
  (4 m-tiles x 6 n-tiles of fp32 matmuls, DVE add folds the positional
  embedding into the mandatory PSUM->SBUF copy), then contiguous DMA out.
The padding mask depends only on the id map and is computed host-side with
numpy, exactly as the reference implementation does.
"""

import numpy as np

PATCH_LENGTHS = (5, 10, 17, 24)
D_MODEL = 512
N_CORES = 8

TRACE = False  # set by test harness to collect a profile
LAST_RESULTS = None  # BassKernelResults of the last device run (for timing)


# --------------------------------------------------------------------------
# Host-side plan (verbatim numpy port of reference._plan)
# --------------------------------------------------------------------------

def _plan(seg_np):
    Bn, N = seg_np.shape
    valid = np.logical_and.accumulate(seg_np != -1, axis=1)
    prev = np.concatenate([np.full((Bn, 1), -2, seg_np.dtype), seg_np[:, :-1]], axis=1)
    starts = valid & (seg_np != prev)
    rows, cols = np.nonzero(starts)
    M = rows.size
    valid_len = valid.sum(1)
    is_last = np.r_[rows[1:] != rows[:-1], np.array([True])]
    next_col = np.r_[cols[1:], np.array([0])]
    lens = np.where(is_last, valid_len[rows] - cols, next_col - cols)
    pl = np.asarray(PATCH_LENGTHS)
    bucket = np.abs(lens[:, None] - pl[None, :]).argmin(1)
    P_arr = pl[bucket]
    row_start = np.searchsorted(rows, np.arange(Bn))
    slot = np.arange(M) - row_start[rows]
    S = M // Bn
    cum = np.cumsum(P_arr)
    row_base = (cum - P_arr)[row_start]
    off = cum - P_arr - row_base[rows]
    total = int(P_arr[row_start[0]:row_start[0] + S].sum())
    return rows, cols, lens, bucket, P_arr, slot, off, S, total


def _pos_embedding_np(seq_len, d_model):
    position = np.arange(seq_len, dtype=np.float32)[:, None]
    div_term = np.exp(
        np.arange(0, d_model, 2, dtype=np.float32) * -(np.log(10000.0) / d_model)
    )
    pe = np.zeros((seq_len, d_model), np.float32)
    pe[:, 0::2] = np.sin(position * div_term)
    pe[:, 1::2] = np.cos(position * div_term)
    return pe


def _mask_from_plan(plan, B):
    rows, cols, lens, bucket, P_arr, slot, off, S, total = plan
    mask = np.zeros((B, total), bool)
    for k, P in enumerate(PATCH_LENGTHS):
        sel = np.nonzero(bucket == k)[0]
        if sel.size == 0:
            continue
        t = np.arange(P)
        padm = t[None, :] >= lens[sel][:, None]
        mask[rows[sel][:, None], off[sel][:, None] + t[None, :]] = padm
    return mask


def _detect_structure(plan, seg):
    """Return (g, n_groups, T, c, L, P, K) if every row has the identical,
    periodic segmentation that admits the one-GEMM rewrite; else None."""
    rows, cols, lens, bucket, P_arr, slot, off, S, total = plan
    B, N = seg.shape
    if S * B != rows.size or not (seg == seg[0]).all():
        return None
    c, L, P, K = cols[:S], lens[:S], P_arr[:S], bucket[:S]
    for g in range(1, S + 1):
        if S % g or N % (S // g):
            continue
        n_groups = S // g
        T = N // n_groups
        if T > 128:
            continue  # single-shot contraction only
        ok = (
            all(c[s] == T * (s // g) + c[s % g] for s in range(S))
            and all(L[s] == L[s % g] for s in range(S))
            and all(P[s] == P[s % g] for s in range(S))
            and all(c[f] + min(L[f], P[f]) <= T for f in range(g))
            and 128 % n_groups == 0
        )
        if ok:
            return g, n_groups, T, c[:g].copy(), L[:g].copy(), P[:g].copy(), K[:g].copy()
    return None


def _numpy_out(x2d, plan, Ws, B):
    """Pure-numpy fallback identical to the reference forward (out only)."""
    rows, cols, lens, bucket, P_arr, slot, off, S, total = plan
    x_flat = x2d.reshape(-1)
    out = np.zeros((B, S, D_MODEL), np.float32)
    N = x2d.shape[1]
    for k, P in enumerate(PATCH_LENGTHS):
        sel = np.nonzero(bucket == k)[0]
        if sel.size == 0:
            continue
        r, cc, L, sl = rows[sel], cols[sel], lens[sel], slot[sel]
        t = np.arange(P)
        padm = t[None, :] >= L[:, None]
        idx = np.where(padm, 0, r[:, None] * N + cc[:, None] + t[None, :])
        vals = np.where(padm, np.float32(0), x_flat[idx])
        out[r, sl] = vals @ Ws[k].T
    return out + _pos_embedding_np(S, D_MODEL)[None]


# --------------------------------------------------------------------------
# Device kernel
# --------------------------------------------------------------------------

_NC_CACHE = {}


def _build_nc(M_core, T, gD):
    import concourse.mybir as mybir
    import concourse.tile as tile
    from concourse import bacc

    f32 = mybir.dt.float32
    nc = bacc.Bacc(
        "TRN2", target_bir_lowering=False, debug=False, enable_asserts=False
    )
    x_in = nc.declare_dram_parameter("x", [M_core, T], f32, isOutput=False)
    w_in = nc.declare_dram_parameter("wbig", [T, gD], f32, isOutput=False)
    pe_in = nc.declare_dram_parameter("pet", [128, gD], f32, isOutput=False)
    id_in = nc.declare_dram_parameter("ident", [128, 128], f32, isOutput=False)
    out = nc.declare_dram_parameter("out", [M_core, gD], f32, isOutput=True)
    n_m = M_core // 128
    n_n = gD // 512
    with tile.TileContext(nc) as tc:
        with (
            tc.tile_pool(name="const", bufs=1) as cpool,
            tc.tile_pool(name="xload", bufs=4) as xpool,
            tc.tile_pool(name="xt", bufs=4) as xtpool,
            tc.tile_pool(name="tp", bufs=2, space="PSUM") as tppool,
            tc.tile_pool(name="mm", bufs=4, space="PSUM") as mmpool,
            tc.tile_pool(name="ot", bufs=3) as opool,
        ):
            w_t = cpool.tile([T, gD], f32)
            nc.sync.dma_start(out=w_t[:], in_=w_in[:])
            pe_t = cpool.tile([128, gD], f32)
            nc.sync.dma_start(out=pe_t[:], in_=pe_in[:])
            id_t = cpool.tile([128, 128], f32)
            nc.sync.dma_start(out=id_t[:], in_=id_in[:])
            for m in range(n_m):
                x_t = xpool.tile([128, T], f32)
                nc.sync.dma_start(out=x_t[:], in_=x_in[m * 128:(m + 1) * 128, :])
                tp = tppool.tile([T, 128], f32)
                nc.tensor.transpose(tp[:], x_t[:], id_t[:])
                xt = xtpool.tile([T, 128], f32)
                nc.vector.tensor_copy(xt[:], tp[:])
                o_t = opool.tile([128, gD], f32)
                for n in range(n_n):
                    sl = slice(n * 512, (n + 1) * 512)
                    ps = mmpool.tile([128, 512], f32)
                    nc.tensor.matmul(ps[:], xt[:], w_t[:, sl], start=True, stop=True)
                    nc.vector.tensor_add(o_t[:, sl], ps[:], pe_t[:, sl])
                nc.scalar.dma_start(out=out[m * 128:(m + 1) * 128, :], in_=o_t[:])
    nc.compile()
    return nc


def _run_device(X, Wbig, PeTile, B, n_groups, g):
    global LAST_RESULTS
    from concourse.bass_utils import run_bass_kernel_spmd

    T = X.shape[1]
    gD = g * D_MODEL
    Bc = B // N_CORES
    M_core = Bc * n_groups
    M_pad = -(-M_core // 128) * 128
    key = (M_pad, T, gD)
    if key not in _NC_CACHE:
        _NC_CACHE[key] = _build_nc(M_pad, T, gD)
    nc = _NC_CACHE[key]

    ident = np.eye(128, dtype=np.float32)
    in_maps = []
    for c in range(N_CORES):
        shard = X[c * M_core:(c + 1) * M_core]
        if M_pad != M_core:
            shard = np.concatenate(
                [shard, np.zeros((M_pad - M_core, T), np.float32)], axis=0
            )
        in_maps.append(
            {
                "x": np.ascontiguousarray(shard),
                "wbig": Wbig,
                "pet": PeTile,
                "ident": ident,
            }
        )
    res = run_bass_kernel_spmd(
        nc, in_maps, list(range(N_CORES)), trace=TRACE
    )
    LAST_RESULTS = res
    outs = [
        res.results[c]["out"][:M_core].reshape(Bc, n_groups * g, D_MODEL)
        for c in range(N_CORES)
    ]
    return np.concatenate(outs, axis=0)


# --------------------------------------------------------------------------
# Entry point
# --------------------------------------------------------------------------

def kernel(x, x_opath_batch, W0, W1, W2, W3):
    x = np.ascontiguousarray(np.asarray(x, dtype=np.float32))
    seg = np.asarray(x_opath_batch)
    Ws = [np.ascontiguousarray(np.asarray(W, dtype=np.float32)) for W in (W0, W1, W2, W3)]
    B, N = seg.shape
    x2d = x.reshape(B, N)

    plan = _plan(seg)
    mask = _mask_from_plan(plan, B)

    st = _detect_structure(plan, seg) if B % N_CORES == 0 else None
    if st is None:
        out = _numpy_out(x2d, plan, Ws, B)
        return out, mask

    g, n_groups, T, c, L, P, K = st
    S = g * n_groups
    gD = g * D_MODEL

    Wbig = np.zeros((T, gD), np.float32)
    for f in range(g):
        eff = int(min(L[f], P[f]))
        Wbig[c[f]:c[f] + eff, f * D_MODEL:(f + 1) * D_MODEL] = Ws[K[f]].T[:eff]

    pe = _pos_embedding_np(S, D_MODEL)  # [S, D]
    PeMat = pe.reshape(n_groups, gD)
    PeTile = np.ascontiguousarray(PeMat[np.arange(128) % n_groups])

    X = x2d.reshape(B * n_groups, T)
    out = _run_device(X, Wbig, PeTile, B, n_groups, g)
    return out, mask


# revision 14
# speedup vs baseline: 1.2270x; 1.2270x over previous
"""Trainium2 Bass kernel for nn_CustomPatchEmbedding.

Math: per row, the int id map segments the 1376 columns into 96 segments.
Each segment becomes one patch: gather min(len, P) values (P = closest of
(5,10,17,24)), multiply by W_P.T -> [512], scatter to out[row, slot], add a
sin/cos positional embedding.  The id map produced by ``setup_inputs`` is
identical across rows and periodic: 6 segments spanning 86 columns, tiled 16
times.  That lets the whole gather + 4 bucketed GEMMs + scatter collapse into
ONE dense GEMM,

    x.reshape(B*16, 86) @ Wbig[86, 6*512]  ->  out.reshape(B, 96, 512)

where Wbig places each W_k.T block at its segment's column offset and encodes
padding/truncation as zero rows.  The structure (period, offsets, buckets) is
re-derived at runtime from the actual ``x_opath_batch`` input; if the input
turns out not to have the expected structure we fall back to a pure-numpy
computation (never triggers for the real harness inputs).

Device work per core (data-parallel over batch, 32 rows/core):
  XT = transpose(x_shard [512, 86]) via PE;  out = XT.T @ Wbig + posemb
  (4 m-tiles x 6 n-tiles of fp32 matmuls, DVE add folds the positional
  embedding into the mandatory PSUM->SBUF copy), then contiguous DMA out.
The padding mask depends only on the id map and is computed host-side with
numpy, exactly as the reference implementation does.
"""

import numpy as np

PATCH_LENGTHS = (5, 10, 17, 24)
D_MODEL = 512
N_CORES = 8

TRACE = False  # set by test harness to collect a profile
LAST_RESULTS = None  # BassKernelResults of the last device run (for timing)
MM_MODE = "f16x3"  # "f32" (4 cyc/col) | "f16x3" (3-term fp16 split, 3 cyc/col)


# --------------------------------------------------------------------------
# Host-side plan (verbatim numpy port of reference._plan)
# --------------------------------------------------------------------------

def _plan(seg_np):
    Bn, N = seg_np.shape
    valid = np.logical_and.accumulate(seg_np != -1, axis=1)
    prev = np.concatenate([np.full((Bn, 1), -2, seg_np.dtype), seg_np[:, :-1]], axis=1)
    starts = valid & (seg_np != prev)
    rows, cols = np.nonzero(starts)
    M = rows.size
    valid_len = valid.sum(1)
    is_last = np.r_[rows[1:] != rows[:-1], np.array([True])]
    next_col = np.r_[cols[1:], np.array([0])]
    lens = np.where(is_last, valid_len[rows] - cols, next_col - cols)
    pl = np.asarray(PATCH_LENGTHS)
    bucket = np.abs(lens[:, None] - pl[None, :]).argmin(1)
    P_arr = pl[bucket]
    row_start = np.searchsorted(rows, np.arange(Bn))
    slot = np.arange(M) - row_start[rows]
    S = M // Bn
    cum = np.cumsum(P_arr)
    row_base = (cum - P_arr)[row_start]
    off = cum - P_arr - row_base[rows]
    total = int(P_arr[row_start[0]:row_start[0] + S].sum())
    return rows, cols, lens, bucket, P_arr, slot, off, S, total


def _pos_embedding_np(seq_len, d_model):
    position = np.arange(seq_len, dtype=np.float32)[:, None]
    div_term = np.exp(
        np.arange(0, d_model, 2, dtype=np.float32) * -(np.log(10000.0) / d_model)
    )
    pe = np.zeros((seq_len, d_model), np.float32)
    pe[:, 0::2] = np.sin(position * div_term)
    pe[:, 1::2] = np.cos(position * div_term)
    return pe


def _mask_from_plan(plan, B):
    rows, cols, lens, bucket, P_arr, slot, off, S, total = plan
    mask = np.zeros((B, total), bool)
    for k, P in enumerate(PATCH_LENGTHS):
        sel = np.nonzero(bucket == k)[0]
        if sel.size == 0:
            continue
        t = np.arange(P)
        padm = t[None, :] >= lens[sel][:, None]
        mask[rows[sel][:, None], off[sel][:, None] + t[None, :]] = padm
    return mask


def _detect_structure(plan, seg):
    """Return (g, n_groups, T, c, L, P, K) if every row has the identical,
    periodic segmentation that admits the one-GEMM rewrite; else None."""
    rows, cols, lens, bucket, P_arr, slot, off, S, total = plan
    B, N = seg.shape
    if S * B != rows.size or not (seg == seg[0]).all():
        return None
    c, L, P, K = cols[:S], lens[:S], P_arr[:S], bucket[:S]
    for g in range(1, S + 1):
        if S % g or N % (S // g):
            continue
        n_groups = S // g
        T = N // n_groups
        if T > 128:
            continue  # single-shot contraction only
        ok = (
            all(c[s] == T * (s // g) + c[s % g] for s in range(S))
            and all(L[s] == L[s % g] for s in range(S))
            and all(P[s] == P[s % g] for s in range(S))
            and all(c[f] + min(L[f], P[f]) <= T for f in range(g))
            and 128 % n_groups == 0
        )
        if ok:
            return g, n_groups, T, c[:g].copy(), L[:g].copy(), P[:g].copy(), K[:g].copy()
    return None


def _numpy_out(x2d, plan, Ws, B):
    """Pure-numpy fallback identical to the reference forward (out only)."""
    rows, cols, lens, bucket, P_arr, slot, off, S, total = plan
    x_flat = x2d.reshape(-1)
    out = np.zeros((B, S, D_MODEL), np.float32)
    N = x2d.shape[1]
    for k, P in enumerate(PATCH_LENGTHS):
        sel = np.nonzero(bucket == k)[0]
        if sel.size == 0:
            continue
        r, cc, L, sl = rows[sel], cols[sel], lens[sel], slot[sel]
        t = np.arange(P)
        padm = t[None, :] >= L[:, None]
        idx = np.where(padm, 0, r[:, None] * N + cc[:, None] + t[None, :])
        vals = np.where(padm, np.float32(0), x_flat[idx])
        out[r, sl] = vals @ Ws[k].T
    return out + _pos_embedding_np(S, D_MODEL)[None]


# --------------------------------------------------------------------------
# Device kernel
# --------------------------------------------------------------------------

_NC_CACHE = {}


def _build_nc(M_core, T, gD, mode="f16x3"):
    import concourse.mybir as mybir
    import concourse.tile as tile
    from concourse import bacc

    f32 = mybir.dt.float32
    f16 = mybir.dt.float16
    split = mode == "f16x3"
    xdt = f16 if split else f32
    nc = bacc.Bacc(
        "TRN2", target_bir_lowering=False, debug=False, enable_asserts=False
    )
    if split:
        xh_in = nc.declare_dram_parameter("xh", [M_core, T], f16, isOutput=False)
        xl_in = nc.declare_dram_parameter("xl", [M_core, T], f16, isOutput=False)
        wh_in = nc.declare_dram_parameter("wh", [T, gD], f16, isOutput=False)
        wl_in = nc.declare_dram_parameter("wl", [T, gD], f16, isOutput=False)
    else:
        x_in = nc.declare_dram_parameter("x", [M_core, T], f32, isOutput=False)
        w_in = nc.declare_dram_parameter("wbig", [T, gD], f32, isOutput=False)
    pe_in = nc.declare_dram_parameter("pet", [128, gD], f32, isOutput=False)
    id_in = nc.declare_dram_parameter("ident", [128, 128], xdt, isOutput=False)
    out = nc.declare_dram_parameter("out", [M_core, gD], f32, isOutput=True)
    n_m = M_core // 128
    n_n = gD // 512
    with tile.TileContext(nc) as tc:
        with (
            tc.tile_pool(name="const", bufs=1) as cpool,
            tc.tile_pool(name="xload", bufs=8) as xpool,
            tc.tile_pool(name="xt", bufs=8) as xtpool,
            tc.tile_pool(name="tp", bufs=2, space="PSUM") as tppool,
            tc.tile_pool(name="mm", bufs=4, space="PSUM") as mmpool,
            tc.tile_pool(name="ot", bufs=3) as opool,
        ):
            # Small loads first so the SP DMA ring serves the transpose
            # inputs before streaming the big weight/posemb tables; W and
            # posemb arrive in per-n-block chunks so (a) descriptors spread
            # across the SDMA engines and (b) the first matmuls start after
            # one chunk instead of the whole table.
            id_t = cpool.tile([128, 128], xdt)
            nc.sync.dma_start(out=id_t[:], in_=id_in[:])
            x_ts = []
            for m in range(n_m):
                msl = slice(m * 128, (m + 1) * 128)
                if split:
                    xh_t = xpool.tile([128, T], f16, tag="x")
                    nc.sync.dma_start(out=xh_t[:], in_=xh_in[msl, :])
                    xl_t = xpool.tile([128, T], f16, tag="x")
                    nc.sync.dma_start(out=xl_t[:], in_=xl_in[msl, :])
                    x_ts.append((xh_t, xl_t))
                else:
                    x_t = xpool.tile([128, T], f32, tag="x")
                    nc.sync.dma_start(out=x_t[:], in_=x_in[msl, :])
                    x_ts.append((x_t,))
            if split:
                wh_t = cpool.tile([T, gD], f16)
                wl_t = cpool.tile([T, gD], f16)
            else:
                w_t = cpool.tile([T, gD], f32)
            pe_t = cpool.tile([128, gD], f32)
            for n in range(n_n):
                sl = slice(n * 512, (n + 1) * 512)
                if split:
                    nc.sync.dma_start(out=wh_t[:, sl], in_=wh_in[:, sl])
                    nc.sync.dma_start(out=wl_t[:, sl], in_=wl_in[:, sl])
                else:
                    nc.sync.dma_start(out=w_t[:, sl], in_=w_in[:, sl])
                nc.sync.dma_start(out=pe_t[:, sl], in_=pe_in[:, sl])
            for m in range(n_m):
                xts = []
                for x_t in x_ts[m]:
                    tp = tppool.tile([T, 128], xdt, tag="tp")
                    nc.tensor.transpose(tp[:], x_t[:], id_t[:])
                    xt = xtpool.tile([T, 128], xdt, tag="xt")
                    nc.vector.tensor_copy(xt[:], tp[:])
                    xts.append(xt)
                o_t = opool.tile([128, gD], f32)
                for n in range(n_n):
                    sl = slice(n * 512, (n + 1) * 512)
                    ps = mmpool.tile([128, 512], f32)
                    if split:
                        xhT, xlT = xts
                        nc.tensor.matmul(
                            ps[:], xhT[:], wh_t[:, sl], start=True, stop=False
                        )
                        nc.tensor.matmul(
                            ps[:], xlT[:], wh_t[:, sl], start=False, stop=False
                        )
                        nc.tensor.matmul(
                            ps[:], xhT[:], wl_t[:, sl], start=False, stop=True
                        )
                    else:
                        nc.tensor.matmul(
                            ps[:], xts[0][:], w_t[:, sl], start=True, stop=True
                        )
                    nc.vector.tensor_add(o_t[:, sl], ps[:], pe_t[:, sl])
                nc.scalar.dma_start(out=out[m * 128:(m + 1) * 128, :], in_=o_t[:])
    nc.compile()
    return nc


def _run_device(X, Wbig, PeTile, B, n_groups, g):
    global LAST_RESULTS
    from concourse.bass_utils import run_bass_kernel_spmd

    T = X.shape[1]
    gD = g * D_MODEL
    Bc = B // N_CORES
    M_core = Bc * n_groups
    M_pad = -(-M_core // 128) * 128
    key = (M_pad, T, gD, MM_MODE)
    if key not in _NC_CACHE:
        _NC_CACHE[key] = _build_nc(M_pad, T, gD, MM_MODE)
    nc = _NC_CACHE[key]

    split = MM_MODE == "f16x3"
    ident = np.eye(128, dtype=np.float16 if split else np.float32)
    if split:
        Wh = Wbig.astype(np.float16)
        Wl = (Wbig - Wh.astype(np.float32)).astype(np.float16)
    in_maps = []
    for c in range(N_CORES):
        shard = X[c * M_core:(c + 1) * M_core]
        if M_pad != M_core:
            shard = np.concatenate(
                [shard, np.zeros((M_pad - M_core, T), np.float32)], axis=0
            )
        shard = np.ascontiguousarray(shard)
        if split:
            xh = shard.astype(np.float16)
            xl = (shard - xh.astype(np.float32)).astype(np.float16)
            in_maps.append(
                {"xh": xh, "xl": xl, "wh": Wh, "wl": Wl, "pet": PeTile,
                 "ident": ident}
            )
        else:
            in_maps.append(
                {"x": shard, "wbig": Wbig, "pet": PeTile, "ident": ident}
            )
    res = run_bass_kernel_spmd(
        nc, in_maps, list(range(N_CORES)), trace=TRACE
    )
    LAST_RESULTS = res
    outs = [
        res.results[c]["out"][:M_core].reshape(Bc, n_groups * g, D_MODEL)
        for c in range(N_CORES)
    ]
    return np.concatenate(outs, axis=0)


# --------------------------------------------------------------------------
# Entry point
# --------------------------------------------------------------------------

def kernel(x, x_opath_batch, W0, W1, W2, W3):
    x = np.ascontiguousarray(np.asarray(x, dtype=np.float32))
    seg = np.asarray(x_opath_batch)
    Ws = [np.ascontiguousarray(np.asarray(W, dtype=np.float32)) for W in (W0, W1, W2, W3)]
    B, N = seg.shape
    x2d = x.reshape(B, N)

    plan = _plan(seg)
    mask = _mask_from_plan(plan, B)

    st = _detect_structure(plan, seg) if B % N_CORES == 0 else None
    if st is None:
        out = _numpy_out(x2d, plan, Ws, B)
        return out, mask

    g, n_groups, T, c, L, P, K = st
    S = g * n_groups
    gD = g * D_MODEL

    Wbig = np.zeros((T, gD), np.float32)
    for f in range(g):
        eff = int(min(L[f], P[f]))
        Wbig[c[f]:c[f] + eff, f * D_MODEL:(f + 1) * D_MODEL] = Ws[K[f]].T[:eff]

    pe = _pos_embedding_np(S, D_MODEL)  # [S, D]
    PeMat = pe.reshape(n_groups, gD)
    PeTile = np.ascontiguousarray(PeMat[np.arange(128) % n_groups])

    X = x2d.reshape(B * n_groups, T)
    out = _run_device(X, Wbig, PeTile, B, n_groups, g)
    return out, mask


# revision 16
# speedup vs baseline: 1.4456x; 1.1781x over previous
"""Trainium2 Bass kernel for nn_CustomPatchEmbedding.

Math: per row, the int id map segments the 1376 columns into 96 segments.
Each segment becomes one patch: gather min(len, P) values (P = closest of
(5,10,17,24)), multiply by W_P.T -> [512], scatter to out[row, slot], add a
sin/cos positional embedding.  The id map produced by ``setup_inputs`` is
identical across rows and periodic: 6 segments spanning 86 columns, tiled 16
times.  That lets the whole gather + 4 bucketed GEMMs + scatter collapse into
ONE dense GEMM,

    x.reshape(B*16, 86) @ Wbig[86, 6*512]  ->  out.reshape(B, 96, 512)

where Wbig places each W_k.T block at its segment's column offset and encodes
padding/truncation as zero rows.  The structure (period, offsets, buckets) is
re-derived at runtime from the actual ``x_opath_batch`` input; if the input
turns out not to have the expected structure we fall back to a pure-numpy
computation (never triggers for the real harness inputs).

Device work per core (data-parallel over batch, 32 rows/core):
  XT = transpose(x_shard [512, 86]) via PE;  out = XT.T @ Wbig + posemb
  (4 m-tiles x 6 n-tiles of fp32 matmuls, DVE add folds the positional
  embedding into the mandatory PSUM->SBUF copy), then contiguous DMA out.
The padding mask depends only on the id map and is computed host-side with
numpy, exactly as the reference implementation does.
"""

import numpy as np

PATCH_LENGTHS = (5, 10, 17, 24)
D_MODEL = 512
N_CORES = 8

TRACE = False  # set by test harness to collect a profile
LAST_RESULTS = None  # BassKernelResults of the last device run (for timing)
MM_MODE = "f16x3"  # "f32" (4 cyc/col) | "f16x3" (3-term fp16 split, 3 cyc/col)


# --------------------------------------------------------------------------
# Host-side plan (verbatim numpy port of reference._plan)
# --------------------------------------------------------------------------

def _plan(seg_np):
    Bn, N = seg_np.shape
    valid = np.logical_and.accumulate(seg_np != -1, axis=1)
    prev = np.concatenate([np.full((Bn, 1), -2, seg_np.dtype), seg_np[:, :-1]], axis=1)
    starts = valid & (seg_np != prev)
    rows, cols = np.nonzero(starts)
    M = rows.size
    valid_len = valid.sum(1)
    is_last = np.r_[rows[1:] != rows[:-1], np.array([True])]
    next_col = np.r_[cols[1:], np.array([0])]
    lens = np.where(is_last, valid_len[rows] - cols, next_col - cols)
    pl = np.asarray(PATCH_LENGTHS)
    bucket = np.abs(lens[:, None] - pl[None, :]).argmin(1)
    P_arr = pl[bucket]
    row_start = np.searchsorted(rows, np.arange(Bn))
    slot = np.arange(M) - row_start[rows]
    S = M // Bn
    cum = np.cumsum(P_arr)
    row_base = (cum - P_arr)[row_start]
    off = cum - P_arr - row_base[rows]
    total = int(P_arr[row_start[0]:row_start[0] + S].sum())
    return rows, cols, lens, bucket, P_arr, slot, off, S, total


def _pos_embedding_np(seq_len, d_model):
    position = np.arange(seq_len, dtype=np.float32)[:, None]
    div_term = np.exp(
        np.arange(0, d_model, 2, dtype=np.float32) * -(np.log(10000.0) / d_model)
    )
    pe = np.zeros((seq_len, d_model), np.float32)
    pe[:, 0::2] = np.sin(position * div_term)
    pe[:, 1::2] = np.cos(position * div_term)
    return pe


def _mask_from_plan(plan, B):
    rows, cols, lens, bucket, P_arr, slot, off, S, total = plan
    mask = np.zeros((B, total), bool)
    for k, P in enumerate(PATCH_LENGTHS):
        sel = np.nonzero(bucket == k)[0]
        if sel.size == 0:
            continue
        t = np.arange(P)
        padm = t[None, :] >= lens[sel][:, None]
        mask[rows[sel][:, None], off[sel][:, None] + t[None, :]] = padm
    return mask


def _detect_structure(plan, seg):
    """Return (g, n_groups, T, c, L, P, K) if every row has the identical,
    periodic segmentation that admits the one-GEMM rewrite; else None."""
    rows, cols, lens, bucket, P_arr, slot, off, S, total = plan
    B, N = seg.shape
    if S * B != rows.size or not (seg == seg[0]).all():
        return None
    c, L, P, K = cols[:S], lens[:S], P_arr[:S], bucket[:S]
    for g in range(1, S + 1):
        if S % g or N % (S // g):
            continue
        n_groups = S // g
        T = N // n_groups
        if T > 128:
            continue  # single-shot contraction only
        ok = (
            all(c[s] == T * (s // g) + c[s % g] for s in range(S))
            and all(L[s] == L[s % g] for s in range(S))
            and all(P[s] == P[s % g] for s in range(S))
            and all(c[f] + min(L[f], P[f]) <= T for f in range(g))
            and 128 % n_groups == 0
        )
        if ok:
            return g, n_groups, T, c[:g].copy(), L[:g].copy(), P[:g].copy(), K[:g].copy()
    return None


def _numpy_out(x2d, plan, Ws, B):
    """Pure-numpy fallback identical to the reference forward (out only)."""
    rows, cols, lens, bucket, P_arr, slot, off, S, total = plan
    x_flat = x2d.reshape(-1)
    out = np.zeros((B, S, D_MODEL), np.float32)
    N = x2d.shape[1]
    for k, P in enumerate(PATCH_LENGTHS):
        sel = np.nonzero(bucket == k)[0]
        if sel.size == 0:
            continue
        r, cc, L, sl = rows[sel], cols[sel], lens[sel], slot[sel]
        t = np.arange(P)
        padm = t[None, :] >= L[:, None]
        idx = np.where(padm, 0, r[:, None] * N + cc[:, None] + t[None, :])
        vals = np.where(padm, np.float32(0), x_flat[idx])
        out[r, sl] = vals @ Ws[k].T
    return out + _pos_embedding_np(S, D_MODEL)[None]


# --------------------------------------------------------------------------
# Device kernel
# --------------------------------------------------------------------------

_NC_CACHE = {}


def _build_nc(M_core, T, gD, mode="f16x3"):
    import concourse.mybir as mybir
    import concourse.tile as tile
    from concourse import bacc

    f32 = mybir.dt.float32
    f16 = mybir.dt.float16
    split = mode == "f16x3"
    xdt = f16 if split else f32
    nc = bacc.Bacc(
        "TRN2", target_bir_lowering=False, debug=False, enable_asserts=False
    )
    if split:
        xh_in = nc.declare_dram_parameter("xh", [M_core, T], f16, isOutput=False)
        xl_in = nc.declare_dram_parameter("xl", [M_core, T], f16, isOutput=False)
        wh_in = nc.declare_dram_parameter("wh", [T, gD], f16, isOutput=False)
        wl_in = nc.declare_dram_parameter("wl", [T, gD], f16, isOutput=False)
    else:
        x_in = nc.declare_dram_parameter("x", [M_core, T], f32, isOutput=False)
        w_in = nc.declare_dram_parameter("wbig", [T, gD], f32, isOutput=False)
    pe_in = nc.declare_dram_parameter("pet", [128, gD], f32, isOutput=False)
    id_in = nc.declare_dram_parameter("ident", [128, 128], xdt, isOutput=False)
    out = nc.declare_dram_parameter("out", [M_core, gD], f32, isOutput=True)
    n_m = M_core // 128
    n_n = gD // 512
    with tile.TileContext(nc) as tc:
        with (
            tc.tile_pool(name="const", bufs=1) as cpool,
            tc.tile_pool(name="xload", bufs=8) as xpool,
            tc.tile_pool(name="xt", bufs=8) as xtpool,
            tc.tile_pool(name="tp", bufs=2, space="PSUM") as tppool,
            tc.tile_pool(name="mm", bufs=4, space="PSUM") as mmpool,
            tc.tile_pool(name="ot", bufs=3) as opool,
        ):
            # Loads go through SWDGE (gpsimd): its per-partition descriptor
            # swizzle spreads every transfer across all 16 SDMA engines —
            # the HWDGE load path was observed to pack big SBUF-dst loads
            # onto 2 engines (~54GB/s), starving the PE.  Small x first so
            # transposes start immediately; W/posemb stream per n-chunk.
            id_t = cpool.tile([128, 128], xdt)
            nc.gpsimd.dma_start(out=id_t[:], in_=id_in[:])
            if split:
                xh_b = xpool.tile([128, n_m * T], f16, tag="x")
                nc.gpsimd.dma_start(
                    out=xh_b[:].rearrange("p (m t) -> p m t", m=n_m),
                    in_=xh_in.rearrange("(m p) t -> p m t", p=128),
                )
                xl_b = xpool.tile([128, n_m * T], f16, tag="x")
                nc.gpsimd.dma_start(
                    out=xl_b[:].rearrange("p (m t) -> p m t", m=n_m),
                    in_=xl_in.rearrange("(m p) t -> p m t", p=128),
                )
                x_bigs = (xh_b, xl_b)
            else:
                x_b = xpool.tile([128, n_m * T], f32, tag="x")
                nc.gpsimd.dma_start(
                    out=x_b[:].rearrange("p (m t) -> p m t", m=n_m),
                    in_=x_in.rearrange("(m p) t -> p m t", p=128),
                )
                x_bigs = (x_b,)
            if split:
                wh_t = cpool.tile([T, gD], f16)
                wl_t = cpool.tile([T, gD], f16)
            else:
                w_t = cpool.tile([T, gD], f32)
            pe_t = cpool.tile([128, gD], f32)
            for n in range(n_n):
                sl = slice(n * 512, (n + 1) * 512)
                if split:
                    nc.gpsimd.dma_start(out=wh_t[:, sl], in_=wh_in[:, sl])
                    nc.gpsimd.dma_start(out=wl_t[:, sl], in_=wl_in[:, sl])
                else:
                    nc.gpsimd.dma_start(out=w_t[:, sl], in_=w_in[:, sl])
                nc.gpsimd.dma_start(out=pe_t[:, sl], in_=pe_in[:, sl])
            for m in range(n_m):
                xts = []
                for x_b in x_bigs:
                    tp = tppool.tile([T, 128], xdt, tag="tp")
                    nc.tensor.transpose(
                        tp[:], x_b[:, m * T:(m + 1) * T], id_t[:]
                    )
                    xt = xtpool.tile([T, 128], xdt, tag="xt")
                    nc.vector.tensor_copy(xt[:], tp[:])
                    xts.append(xt)
                o_t = opool.tile([128, gD], f32)
                for n in range(n_n):
                    sl = slice(n * 512, (n + 1) * 512)
                    ps = mmpool.tile([128, 512], f32)
                    if split:
                        xhT, xlT = xts
                        nc.tensor.matmul(
                            ps[:], xhT[:], wh_t[:, sl], start=True, stop=False
                        )
                        nc.tensor.matmul(
                            ps[:], xlT[:], wh_t[:, sl], start=False, stop=False
                        )
                        nc.tensor.matmul(
                            ps[:], xhT[:], wl_t[:, sl], start=False, stop=True
                        )
                    else:
                        nc.tensor.matmul(
                            ps[:], xts[0][:], w_t[:, sl], start=True, stop=True
                        )
                    nc.vector.tensor_add(o_t[:, sl], ps[:], pe_t[:, sl])
                    if n % 2 == 1:
                        # store in 512KB chunks: spreads store traffic through
                        # the kernel and keeps the final store (and thus the
                        # tail) small
                        osl = slice((n - 1) * 512, (n + 1) * 512)
                        nc.scalar.dma_start(
                            out=out[m * 128:(m + 1) * 128, osl],
                            in_=o_t[:, osl],
                        )
    nc.compile()
    return nc


def _run_device(X, Wbig, PeTile, B, n_groups, g):
    global LAST_RESULTS
    from concourse.bass_utils import run_bass_kernel_spmd

    T = X.shape[1]
    gD = g * D_MODEL
    Bc = B // N_CORES
    M_core = Bc * n_groups
    M_pad = -(-M_core // 128) * 128
    key = (M_pad, T, gD, MM_MODE)
    if key not in _NC_CACHE:
        _NC_CACHE[key] = _build_nc(M_pad, T, gD, MM_MODE)
    nc = _NC_CACHE[key]

    split = MM_MODE == "f16x3"
    ident = np.eye(128, dtype=np.float16 if split else np.float32)
    if split:
        Wh = Wbig.astype(np.float16)
        Wl = (Wbig - Wh.astype(np.float32)).astype(np.float16)
    in_maps = []
    for c in range(N_CORES):
        shard = X[c * M_core:(c + 1) * M_core]
        if M_pad != M_core:
            shard = np.concatenate(
                [shard, np.zeros((M_pad - M_core, T), np.float32)], axis=0
            )
        shard = np.ascontiguousarray(shard)
        if split:
            xh = shard.astype(np.float16)
            xl = (shard - xh.astype(np.float32)).astype(np.float16)
            in_maps.append(
                {"xh": xh, "xl": xl, "wh": Wh, "wl": Wl, "pet": PeTile,
                 "ident": ident}
            )
        else:
            in_maps.append(
                {"x": shard, "wbig": Wbig, "pet": PeTile, "ident": ident}
            )
    res = run_bass_kernel_spmd(
        nc, in_maps, list(range(N_CORES)), trace=TRACE
    )
    LAST_RESULTS = res
    outs = [
        res.results[c]["out"][:M_core].reshape(Bc, n_groups * g, D_MODEL)
        for c in range(N_CORES)
    ]
    return np.concatenate(outs, axis=0)


# --------------------------------------------------------------------------
# Entry point
# --------------------------------------------------------------------------

def kernel(x, x_opath_batch, W0, W1, W2, W3):
    x = np.ascontiguousarray(np.asarray(x, dtype=np.float32))
    seg = np.asarray(x_opath_batch)
    Ws = [np.ascontiguousarray(np.asarray(W, dtype=np.float32)) for W in (W0, W1, W2, W3)]
    B, N = seg.shape
    x2d = x.reshape(B, N)

    plan = _plan(seg)
    mask = _mask_from_plan(plan, B)

    st = _detect_structure(plan, seg) if B % N_CORES == 0 else None
    if st is None:
        out = _numpy_out(x2d, plan, Ws, B)
        return out, mask

    g, n_groups, T, c, L, P, K = st
    S = g * n_groups
    gD = g * D_MODEL

    Wbig = np.zeros((T, gD), np.float32)
    for f in range(g):
        eff = int(min(L[f], P[f]))
        Wbig[c[f]:c[f] + eff, f * D_MODEL:(f + 1) * D_MODEL] = Ws[K[f]].T[:eff]

    pe = _pos_embedding_np(S, D_MODEL)  # [S, D]
    PeMat = pe.reshape(n_groups, gD)
    PeTile = np.ascontiguousarray(PeMat[np.arange(128) % n_groups])

    X = x2d.reshape(B * n_groups, T)
    out = _run_device(X, Wbig, PeTile, B, n_groups, g)
    return out, mask
